# revision 87
# baseline (speedup 1.0000x reference)
"""Trainium2 Bass kernel for nn_Attention_91225105367483.

Spatial attention: x[B=2,T=8,H=32,W=32,D=768] -> 16 frames x 1024 tokens.
Data-parallel over frames: 8 cores x 2 frames each. No collectives.

v5 design (v3 + PE offloads + pipeline/tail restructuring; 344us -> 306us
in timeline-sim):
  - all matmuls bf16 (fp8 measured too coarse for the 2% gate).
  - scores transposed [keys, queries], 2 heads packed into the PE array via
    tile_position quadrants (64-row contraction each).
  - exp on ACT with scale=1/8, bias=-2 (softmax-invariant shift).
  - softmax denominators ride row 64 of the AV matmul (ones column in v);
    normalize: den row -> [1,QCS] copy -> reciprocal -> GPSIMD
    partition_broadcast -> two aligned per-head muls. No PE involvement.
    The last block's evac copies go to ACT (idle then) off the DVE chain.
  - RoPE: q/k head dims are host-permuted into [even(32)|odd(32)] blocks
    per head (score contraction is permutation-invariant), so rotate-half
    becomes four partition-block swap DMAs per tile; the sign lives in the
    sin table. The cos-mul runs on idle GPSIMD, sin-mul + add on DVE.
  - emission interleaves frame f+1's QKV/RoPE into frame f's attention
    blocks so the in-order PE queue always has ready work while ACT runs
    exp; AV lags its exp by 5 key tiles so a not-yet-ready AV never
    head-of-line-blocks the PE queue.
  - cold start: x/w DMAs are chunked and ordered against the serial
    HWDGE/DMA line (x j0,j1 small first; per-j wv/wq; wk 2x3j; cos/sin/
    wo6/bias deferred past the head phase); the chunk-0 v projection is
    emitted j-major across 8 borrowed PSUM regions so each (x_j, wv_j)
    arrival feeds 3072 output cols; separate v/q/k weight tiles keep
    reads from over-waiting on unrelated weight writes; frame-0 qk evacs
    run on ACT (DVE would backlog and hold PSUM banks).
  - tail: outproj(1) st4-7 runs as 8 parallel chains over all PSUM banks
    with the j=5 (last pair) matmul deferred to the end of each chain, the
    output bias folded in as a K=1 ones-row matmul, evac copies split
    DVE/ACT, and one merged flush DMA per s-tile; the weight pool is
    released mid-kernel (LIFO heap) to fund a 6-deep osb pool so flushes
    never stall on buffer rotation.
"""
import sys

sys.path.insert(0, "/opt/trn_rl_repo")

import numpy as np
import ml_dtypes

BF16 = ml_dtypes.bfloat16

B, T, D = 2, 8, 768
NH, HD = 12, 64
NCORES = 8
FPC = 2  # frames per core
NPAIR = NH // 2

GP_ROPE = True  # offload rope cos-mul to GPSIMD

_built = {}


def _host_rope(H, W, head_dim):
    """Replicates reference._rope_cos_sin in numpy fp32."""
    half = head_dim // 4
    inv_freq = (1.0 / (10000.0 ** (np.arange(half, dtype=np.float32) / half))).astype(
        np.float32
    )
    th_h = np.arange(H, dtype=np.float32)[:, None] * inv_freq  # [H, half]
    th_w = np.arange(W, dtype=np.float32)[:, None] * inv_freq  # [W, half]
    cos = np.concatenate(
        [
            np.broadcast_to(np.cos(th_h)[:, None, :], (H, W, half)),
            np.broadcast_to(np.cos(th_w)[None, :, :], (H, W, half)),
        ],
        axis=-1,
    )
    sin = np.concatenate(
        [
            np.broadcast_to(np.sin(th_h)[:, None, :], (H, W, half)),
            np.broadcast_to(np.sin(th_w)[None, :, :], (H, W, half)),
        ],
        axis=-1,
    )
    cos = np.repeat(cos, 2, axis=-1).reshape(H * W, head_dim).astype(np.float32)
    sin = np.repeat(sin, 2, axis=-1).reshape(H * W, head_dim).astype(np.float32)
    return cos, sin


# new row i<32 holds old (even) dim 2i; row 32+i holds old (odd) dim 2i+1
PERM64 = np.concatenate([np.arange(0, 64, 2), np.arange(1, 64, 2)])


def build_nc(H, W):
    """Builds the per-core Bass program. S = H*W tokens per frame."""
    import concourse.bass as bass
    import concourse.tile as tile
    from concourse import bacc, mybir

    dt = mybir.dt
    ActF = mybir.ActivationFunctionType
    S = H * W  # 1024 tokens per frame
    SL = FPC * S  # 2048 tokens per core
    QCS = 512  # query chunk
    NQC = S // QCS  # 2
    KT = S // 128  # 8 key tiles per frame
    NDC = D // 128  # 6 contraction chunks

    nc = bacc.Bacc("TRN2", target_bir_lowering=False, debug=False)

    x6d = nc.dram_tensor("x6", [128, NDC * SL], dt.bfloat16, kind="ExternalInput")
    w6d = nc.dram_tensor("w6", [128, NDC * 3 * D], dt.bfloat16, kind="ExternalInput")
    wo6d = nc.dram_tensor("wo6", [128, NDC * D], dt.bfloat16, kind="ExternalInput")
    cosP = nc.dram_tensor("cosP", [128, S], dt.bfloat16, kind="ExternalInput")
    sinP = nc.dram_tensor("sinP", [128, S], dt.bfloat16, kind="ExternalInput")
    bias_rep = nc.dram_tensor("bias_rep", [128, D], dt.bfloat16, kind="ExternalInput")
    out = nc.dram_tensor("out", [SL, D], dt.float32, kind="ExternalOutput")

    with tile.TileContext(nc) as tc:
        import contextlib
        import itertools

        ctx = contextlib.ExitStack()
        with ctx:
            const = ctx.enter_context(tc.tile_pool(name="const", bufs=1))
            xt_pool = ctx.enter_context(tc.tile_pool(name="xt", bufs=3))
            qk_pool = ctx.enter_context(tc.tile_pool(name="qk", bufs=1))
            swp_pool = ctx.enter_context(tc.tile_pool(name="swp", bufs=1))
            v_pool = ctx.enter_context(tc.tile_pool(name="v", bufs=1))
            ot_pool = ctx.enter_context(tc.tile_pool(name="ot", bufs=1))
            et_pool = ctx.enter_context(tc.tile_pool(name="et", bufs=7))
            rtmp_pool = ctx.enter_context(tc.tile_pool(name="rtmp", bufs=4))
            nrm_pool = ctx.enter_context(tc.tile_pool(name="nrm", bufs=2))
            osba_pool = ctx.enter_context(tc.tile_pool(name="osba", bufs=2))
            mm_ps = ctx.enter_context(tc.tile_pool(name="mmps", bufs=2, space="PSUM"))
            sc_ps = ctx.enter_context(tc.tile_pool(name="scps", bufs=2, space="PSUM"))
            av_ps = ctx.enter_context(tc.tile_pool(name="avps", bufs=2, space="PSUM"))
            # qkv weights live in a manually-released pool: dead after the
            # last frame-1 projection, their 27.6KB then funds a deep osb
            # pool for the outproj/tail flushes (pool heap is LIFO)
            w6p_pool = tc.alloc_tile_pool(name="w6p", bufs=1)
            pools = {"osb": osba_pool}

            frames = [{} for _ in range(FPC)]
            x6v = x6d[:].rearrange("p (j t) -> p j t", j=NDC)

            # ---- constants ----
            # DMA order matters: HWDGE serializes all DMAs (~625ns apiece)
            # and DMA_ENGINES serializes transfers, so arrivals must track PE
            # consumption: (x_j, wv_j) pairs first, then per-j wq/wk, then
            # cos/sin, wo6/bias last.
            xt00 = xt_pool.tile([128, NDC * QCS], dt.bfloat16, tag="xt",
                                name="xt_0_0")
            frames[0]["xt0"] = xt00
            # separate v/q/k weight tiles: a read then only depends on its
            # own tile's DMAs (a single shared tile made every qk chain wait
            # for the LAST of the 14 weight writes)
            w6v_t = w6p_pool.tile([128, NDC * D], dt.bfloat16, tag="w6v")
            w6q_t = w6p_pool.tile([128, NDC * D], dt.bfloat16, tag="w6q")
            w6k_t = w6p_pool.tile([128, NDC * D], dt.bfloat16, tag="w6k")
            w63s = w6d[:].rearrange("p (j e) -> p j e", j=NDC)
            w6vv = w6v_t[:].rearrange("p (j e) -> p j e", j=NDC)
            w6qv = w6q_t[:].rearrange("p (j e) -> p j e", j=NDC)
            w6kv = w6k_t[:].rearrange("p (j e) -> p j e", j=NDC)
            # x chunk 0 as j0, j1, then j2-5 merged: the first matmul waits
            # only on a small j0 transfer, later j's amortize HWDGE slots
            xt00v = xt00[:].rearrange("p (j t) -> p j t", j=NDC)
            for h0, hn in ((0, 1), (1, 1), (2, 4)):
                nc.sync.dma_start(
                    xt00v[:, h0 : h0 + hn, :], x6v[:, h0 : h0 + hn, 0:QCS]
                )
            for j in range(NDC):
                nc.scalar.dma_start(
                    w6vv[:, j, :], w63s[:, j, 2 * D : 3 * D]
                )
            for h0 in (0, 2, 4):  # q columns in 2j chunks (consumed later)
                nc.scalar.dma_start(
                    w6qv[:, h0 : h0 + 2, :], w63s[:, h0 : h0 + 2, 0:D]
                )
            for h0 in (0, 3):  # k columns in two 3-j chunks (consumed later)
                nc.scalar.dma_start(
                    w6kv[:, h0 : h0 + 3, :],
                    w63s[:, h0 : h0 + 3, D : 2 * D],
                )
            # cos/sin/wo6/bias DMAs are deferred until after the head phase
            # so their transfers don't delay wq/wk in the serial DMA line
            cos_t = const.tile([128, S], dt.bfloat16, tag="cos")
            sin_t = const.tile([128, S], dt.bfloat16, tag="sin")
            wo6_t = const.tile([128, NDC * D], dt.bfloat16, tag="wo6")
            bias_t = const.tile([128, D], dt.bfloat16, tag="bias")
            # K=1 ones row: folds the output bias into each tail outproj
            # chain as one extra matmul (PE idles at the tail; DVE doesn't)
            ones_t = const.tile([1, 128], dt.bfloat16, tag="onesr")
            nc.vector.memset(ones_t[:], 1.0)

            # per-partition bias operand for exp(s*scale - 2): keeps et well
            # inside bf16 range; softmax ratio is invariant to the shift
            expb_t = const.tile([128, 1], dt.float32, tag="expb")
            nc.vector.memset(expb_t[:], -2.0)

            wo6 = wo6_t[:].rearrange("p (j e) -> p j e", j=NDC)

            scale = 1.0 / np.sqrt(HD)

            def make_frame_tiles(f):
                fr = frames[f]
                fr["qk_q"] = [
                    qk_pool.tile([128, S], dt.bfloat16, tag=f"q{p}", bufs=2,
                                 name=f"qkq{f}_{p}")
                    for p in range(NPAIR)
                ]
                fr["qk_k"] = [
                    qk_pool.tile([128, S], dt.bfloat16, tag=f"k{p}", bufs=2,
                                 name=f"qkk{f}_{p}")
                    for p in range(NPAIR)
                ]
                fr["v_sb"] = [
                    v_pool.tile([128, NH * 65], dt.bfloat16, tag=f"v{i}", bufs=2,
                                name=f"vsb{f}_{i}")
                    for i in range(KT)
                ]
                fr["ot"] = [
                    ot_pool.tile([128, S], dt.bfloat16, tag=f"ot{d}", bufs=2,
                                 name=f"ot{f}_{d}")
                    for d in range(NDC)
                ]
                for i in range(KT):
                    vv = fr["v_sb"][i][:].rearrange("p (h c) -> p h c", h=NH)
                    nc.vector.memset(vv[:, :, HD : HD + 1], 1.0)

            # ---- emission-level software pipeline ----

            def emit_x(f, c, eng=None):
                """One x-chunk DMA on a queue that won't contend with the
                startup weight stream (HWDGE serializes all DMAs)."""
                fr = frames[f]
                xt = xt_pool.tile([128, NDC * QCS], dt.bfloat16, tag="xt",
                                  name=f"xt_{f}_{c}")
                t0 = f * S + c * QCS
                (eng or nc.sync).dma_start(
                    xt[:].rearrange("p (j t) -> p j t", j=NDC),
                    x6v[:, :, t0 : t0 + QCS],
                )
                fr[f"xt{c}"] = xt

            def qkv_groups(f, part, skip_x0=False):
                """Yields closures, one matmul-group each, for frame f's qkv
                projection (v in natural layout, q/k transposed). part="head"
                yields x DMAs + v + pair 0's q/k/rope (everything the first
                attention block of the frame needs); part="rest" yields pairs
                1..5, each pair's q/k followed by its swap DMAs + rope."""

                def mk_v(c, st):
                    def emit_v(c=c, st=st, f=f):
                        fr = frames[f]
                        kt = c * 4 + st
                        x3 = fr[f"xt{c}"][:].rearrange("p (j t) -> p j t", j=NDC)
                        for nch in range(2):
                            n0 = nch * 512
                            nw = 512 - nch * 256  # v cols: 512 + 256
                            ps = mm_ps.tile([128, 512], dt.float32, tag="mm",
                                            name=f"vps_{f}_{kt}_{nch}")
                            for j in range(NDC):
                                nc.tensor.matmul(
                                    ps[:, 0:nw],
                                    x3[:, j, st * 128 : (st + 1) * 128],
                                    w6vv[:, j, n0 : n0 + nw],
                                    start=(j == 0),
                                    stop=(j == NDC - 1),
                                )
                            vh = fr["v_sb"][kt][:].rearrange(
                                "p (h c) -> p h c", h=NH
                            )
                            pv = ps[:, 0:nw].rearrange("p (h c) -> p h c", c=HD)
                            h0 = nch * 8
                            nh = 8 - nch * 4
                            if f == 0:
                                nc.scalar.copy(vh[:, h0 : h0 + nh, 0:HD], pv[:])
                            else:
                                nc.vector.tensor_copy(
                                    vh[:, h0 : h0 + nh, 0:HD], pv[:])

                    return emit_v

                def mk_qk(c, et):
                    def emit_qk(c=c, et=et, f=f):
                        fr = frames[f]
                        x3 = fr[f"xt{c}"][:].rearrange("p (j t) -> p j t", j=NDC)
                        ps = mm_ps.tile([128, 512], dt.float32, tag="mm",
                                        name=f"qkps_{f}_{c}_{et}")
                        wsl = w6qv if et < 6 else w6kv
                        e0 = (et % 6) * 128
                        for j in range(NDC):
                            nc.tensor.matmul(
                                ps[:],
                                wsl[:, j, e0 : e0 + 128],
                                x3[:, j, :],
                                start=(j == 0),
                                stop=(j == NDC - 1),
                            )
                        dst = fr["qk_q"][et] if et < 6 else fr["qk_k"][et - 6]
                        if f == 0:
                            nc.scalar.copy(dst[:, c * QCS : (c + 1) * QCS], ps[:])
                        else:
                            nc.vector.tensor_copy(
                                dst[:, c * QCS : (c + 1) * QCS], ps[:])

                    return emit_qk

                def emit_v_cold(f=f):
                    """Cold-start v projection for chunk 0, j-major across
                    eight PSUM regions (borrowing the idle av and sc banks):
                    each (x_j, wv_j) DMA arrival feeds all 8 chain steps
                    (3072 output cols ~= 1.28us), matching PE consumption to
                    the ~1.25us/pair serial HWDGE cadence."""
                    fr = frames[f]
                    x3 = fr["xt0"][:].rearrange("p (j t) -> p j t", j=NDC)
                    ps_n0 = [
                        mm_ps.tile([128, 512], dt.float32, tag="mm",
                                   name=f"vcold_m{st}")
                        for st in range(2)
                    ] + [
                        av_ps.tile([128, 512], dt.float32, tag="av",
                                   name=f"vcold_a{st}")
                        for st in range(2)
                    ]
                    sc_a = sc_ps.tile([128, 2 * QCS], dt.float32, tag="sc",
                                      name="vcold_sc0")
                    sc_b = sc_ps.tile([128, 2 * QCS], dt.float32, tag="sc",
                                      name="vcold_sc1")
                    ps_n1 = [sc_a[:, 0:256], sc_a[:, 512:768],
                             sc_b[:, 0:256], sc_b[:, 512:768]]
                    for j in range(NDC):
                        for st in range(4):
                            nc.tensor.matmul(
                                ps_n0[st][:],
                                x3[:, j, st * 128 : (st + 1) * 128],
                                w6vv[:, j, 0:512],
                                start=(j == 0),
                                stop=(j == NDC - 1),
                            )
                        for st in range(4):
                            nc.tensor.matmul(
                                ps_n1[st],
                                x3[:, j, st * 128 : (st + 1) * 128],
                                w6vv[:, j, 512:D],
                                start=(j == 0),
                                stop=(j == NDC - 1),
                            )
                    for st in range(4):
                        vh = fr["v_sb"][st][:].rearrange("p (h c) -> p h c", h=NH)
                        pv = ps_n0[st][:].rearrange("p (h c) -> p h c", c=HD)
                        nc.vector.tensor_copy(vh[:, 0:8, 0:HD], pv[:])
                        pv1 = ps_n1[st].rearrange("p (h c) -> p h c", c=HD)
                        nc.vector.tensor_copy(vh[:, 8:12, 0:HD], pv1[:])

                if part == "head":
                    # chunk 0: x, v, all 12 q/k tiles; chunk 1: x, v
                    if f != 0 and not skip_x0:
                        yield lambda: emit_x(f, 0)
                    if f == 0:
                        yield emit_v_cold
                    else:
                        for st in range(4):
                            yield mk_v(0, st)
                    for et in range(12):
                        yield mk_qk(0, et)
                    yield lambda: emit_x(f, 1, eng=nc.scalar)
                    for st in range(4):
                        yield mk_v(1, st)
                else:
                    # chunk 1 q/k with each pair's swap + rope woven in after
                    for p in part:
                        yield mk_qk(1, p)
                        yield mk_qk(1, 6 + p)
                        yield from rope_groups(f, pairs=[p])

            def rope_groups(f, pairs=None):
                """Yields closures: first the pair's rotate-half swap DMAs,
                then one rope chunk (pair, q-or-k, half-frame) each."""
                for p in pairs if pairs is not None else range(NPAIR):
                    def emit_swap(p=p, f=f):
                        fr = frames[f]
                        for which in range(2):
                            tens = (fr["qk_q"] if which == 0 else fr["qk_k"])[p]
                            swp = swp_pool.tile([128, S], dt.bfloat16,
                                                tag="sw", bufs=2,
                                                name=f"swp_{f}_{p}_{which}")
                            fr[f"swp{which}_{p}"] = swp
                            for b0, b1 in ((0, 32), (64, 96)):
                                nc.sync.dma_start(
                                    swp[b0 : b0 + 32, :], tens[b1 : b1 + 32, :]
                                )
                                nc.sync.dma_start(
                                    swp[b1 : b1 + 32, :], tens[b0 : b0 + 32, :]
                                )

                    yield emit_swap
                    for which in range(2):
                        for c in range(2):
                            def emit_rope(p=p, which=which, c=c, f=f):
                                fr = frames[f]
                                tens = (fr["qk_q"] if which == 0 else fr["qk_k"])[p]
                                swp = fr[f"swp{which}_{p}"]
                                sl_ = slice(c * 512, (c + 1) * 512)
                                t1 = rtmp_pool.tile([128, 512], dt.bfloat16, tag="rt1",
                                                    name=f"rt1_{f}_{p}_{which}_{c}")
                                nc.vector.tensor_mul(t1[:], swp[:, sl_], sin_t[:, sl_])
                                t2 = rtmp_pool.tile([128, 512], dt.bfloat16, tag="rt2",
                                                    name=f"rt2_{f}_{p}_{which}_{c}")
                                # t2 is off the critical path -> idle GPSIMD;
                                # the add gates scores -> keep it on DVE
                                eng = nc.gpsimd if GP_ROPE else nc.vector
                                eng.tensor_mul(t2[:], tens[:, sl_], cos_t[:, sl_])
                                nc.vector.tensor_add(tens[:, sl_], t1[:], t2[:])

                            yield emit_rope

            def attn_block(f, p, qc, filler, last=False):
                """Emits attention for (frame, pair, query-chunk), pulling
                groups from `filler` so the PE queue has independent work.
                For the final block the PSUM-evac copies go to the idle ACT
                engine so the tail-gating normalize isn't DVE-serial."""
                fr = frames[f]
                qsl = slice(qc * QCS, (qc + 1) * QCS)
                avp = [
                    av_ps.tile([128, QCS], dt.float32, tag="av",
                               name=f"avp_{f}_{p}_{qc}_{hh}")
                    for hh in range(2)
                ]

                def emit_av(kt, et_t):
                    for hh in range(2):
                        h = 2 * p + hh
                        nc.tensor.matmul(
                            avp[hh][0:65, :],
                            fr["v_sb"][kt][:, h * 65 : h * 65 + 65],
                            et_t[:, hh * QCS : (hh + 1) * QCS],
                            start=(kt == 0),
                            stop=(kt == KT - 1),
                        )

                pend = []  # AV lags five key tiles behind exp (queue-head safety)
                for kt in range(KT):
                    et_t = et_pool.tile([128, 2 * QCS], dt.bfloat16, tag="et",
                                        name=f"et_{f}_{p}_{qc}_{kt}")
                    ksl = slice(kt * 128, (kt + 1) * 128)
                    sp = sc_ps.tile([128, 2 * QCS], dt.float32, tag="sc",
                                    name=f"sp_{f}_{p}_{qc}_{kt}")
                    for hh in range(2):
                        rb = 64 * hh
                        nc.tensor.matmul(
                            sp[:, hh * QCS : (hh + 1) * QCS],
                            fr["qk_k"][p][rb : rb + 64, ksl],
                            fr["qk_q"][p][rb : rb + 64, qsl],
                            start=True,
                            stop=True,
                            tile_position=(rb, 0),
                        )
                    nc.scalar.activation(
                        et_t[:], sp[:], ActF.Exp, bias=expb_t[:], scale=float(scale)
                    )
                    pend.append((kt, et_t))
                    if len(pend) > 5:
                        emit_av(*pend.pop(0))
                        g = next(filler, None)
                        if g:
                            g()
                for item in pend:
                    emit_av(*item)

                # ---- normalize: o[0:64] / o[64] for both heads ----
                # evacuate the av PSUM banks immediately (den rows + o rows to
                # SBUF); reciprocal on the [1,QCS] row, GPSIMD broadcasts it
                # across partitions, then one aligned mul per head.
                ou = nrm_pool.tile([128, QCS], dt.bfloat16, tag="ou",
                                   name=f"ou_{f}_{p}_{qc}")
                rbcs = []
                for hh in range(2):
                    d_t = nrm_pool.tile([1, QCS], dt.float32, tag=f"den{hh}",
                                        name=f"den_{f}_{p}_{qc}_{hh}")
                    if last:
                        nc.scalar.copy(d_t[:], avp[hh][64:65, :])
                    else:
                        nc.vector.tensor_copy(d_t[:], avp[hh][64:65, :])
                    nc.vector.reciprocal_approx_fast(d_t[:], d_t[:])
                    rbc = nrm_pool.tile([128, QCS], dt.float32, tag=f"rbc{hh}",
                                        name=f"rbc_{f}_{p}_{qc}_{hh}")
                    nc.gpsimd.partition_broadcast(rbc[:], d_t[:], channels=128)
                    rbcs.append(rbc)
                    if last:
                        nc.scalar.copy(
                            ou[64 * hh : 64 * hh + 64, :], avp[hh][0:64, :]
                        )
                    else:
                        nc.vector.tensor_copy(
                            ou[64 * hh : 64 * hh + 64, :], avp[hh][0:64, :]
                        )
                otd = fr["ot"][p]
                nc.vector.tensor_mul(
                    otd[0:64, qsl], ou[0:64, :], rbcs[0][0:64, :]
                )
                nc.vector.tensor_mul(
                    otd[64:128, qsl], ou[64:128, :], rbcs[1][64:128, :]
                )

            def op_flush(f, st, osb):
                """One merged out-DMA per s-tile (HWDGE serializes DMAs, so
                fewer/bigger beats per-chunk at the tail)."""
                T0 = f * S
                nc.sync.dma_start(
                    out[T0 + st * 128 : T0 + (st + 1) * 128, :], osb[:]
                )

            def op_osb(f, st):
                return pools["osb"].tile([128, D], dt.float32, tag="osb",
                                         name=f"osb_{f}_{st}")

            def outproj_groups(f, sts=None):
                """Yields closures: one out-projection column chunk per group
                (finer granularity spreads filler over more pull slots)."""
                sts = list(sts if sts is not None else range(KT))
                for st in sts:
                    box = {}

                    def mk_chunk(st, n0, n1, box, last):
                        def emit_chunk(st=st, n0=n0, n1=n1, f=f):
                            fr = frames[f]
                            if "osb" not in box:
                                box["osb"] = op_osb(f, st)
                            ps = mm_ps.tile([128, 512], dt.float32, tag="mm",
                                            name=f"ops_{f}_{st}_{n0}")
                            for j in range(NDC):
                                nc.tensor.matmul(
                                    ps[:, : n1 - n0],
                                    fr["ot"][j][:, st * 128 : (st + 1) * 128],
                                    wo6[:, j, n0:n1],
                                    start=(j == 0),
                                    stop=(j == NDC - 1),
                                )
                            nc.vector.tensor_add(
                                box["osb"][:, n0:n1], ps[:, : n1 - n0],
                                bias_t[:, n0:n1],
                            )
                            if last:
                                op_flush(f, st, box["osb"])

                        return emit_chunk

                    yield mk_chunk(st, 0, 512, box, False)
                    yield mk_chunk(st, 512, D, box, True)

            def tail_wave():
                """outproj(1) st 4-7 at drain time. The j contraction runs
                over pair index, so j0-4 of all eight chains (2 mm + 2x2 sc
                + 2 av PSUM regions) execute while the last pair's normalize
                is still in flight; each chain's j=5 then completes with its
                evac immediately behind it so the out-DMAs pipeline under
                the remaining PE work."""
                fr = frames[1]
                specs = [(4, 0, 512), (4, 512, D), (5, 0, 512), (5, 512, D),
                         (6, 0, 512), (6, 512, D), (7, 0, 512), (7, 512, D)]
                chans = []
                sc_tiles = {}
                for st, n0, n1 in specs:
                    if st == 4:
                        ps = mm_ps.tile([128, 512], dt.float32, tag="mm",
                                        name=f"tw_{st}_{n0}")[:, : n1 - n0]
                    elif st == 7:
                        # av banks: their WAR (last block's ou copies) clears
                        # early in the wave, and st7's j5 gates on the same
                        # normalize anyway
                        ps = av_ps.tile([128, QCS], dt.float32, tag="av",
                                        name=f"tw_{st}_{n0}")[:, : n1 - n0]
                    else:
                        if st not in sc_tiles:
                            sc_tiles[st] = sc_ps.tile(
                                [128, 2 * QCS], dt.float32, tag="sc",
                                name=f"twsc_{st}")
                        ps = sc_tiles[st][:, n0 : n0 + (n1 - n0)]
                    chans.append(ps)
                # st4's mm banks carry no live WAR: its bias matmul opens
                # the chain here, off the post-norm path to the first flush
                for idx, (st, n0, n1) in enumerate(specs):
                    if st == 4:
                        nc.tensor.matmul(
                            chans[idx], ones_t[:], bias_t[0:1, n0:n1],
                            start=True, stop=False,
                        )
                # st7's av banks have a WAR on the last block's norm copies:
                # emit those chains after the other 30 matmuls so the park
                # never reaches the queue head while the WAR is live
                for j in range(5):
                    for idx, (st, n0, n1) in enumerate(specs):
                        if st == 7:
                            continue
                        nc.tensor.matmul(
                            chans[idx],
                            fr["ot"][j][:, st * 128 : (st + 1) * 128],
                            wo6[:, j, n0:n1],
                            start=(j == 0 and st != 4),
                            stop=False,
                        )
                for j in range(5):
                    for idx, (st, n0, n1) in enumerate(specs):
                        if st != 7:
                            continue
                        nc.tensor.matmul(
                            chans[idx],
                            fr["ot"][j][:, st * 128 : (st + 1) * 128],
                            wo6[:, j, n0:n1],
                            start=(j == 0),
                            stop=False,
                        )
                T0 = S
                for st in (4, 5, 6, 7):
                    osb = op_osb(1, st)
                    for idx, (st_, n0, n1) in enumerate(specs):
                        if st_ != st:
                            continue
                        nc.tensor.matmul(
                            chans[idx],
                            fr["ot"][5][:, st * 128 : (st + 1) * 128],
                            wo6[:, 5, n0:n1],
                            start=False,
                            stop=(st == 4),
                        )
                        # bias via a K=1 ones-row matmul (st4's opened its
                        # chain instead), then evac copies alternating
                        # DVE / idle ACT; each chunk's flush DMA issues
                        # right behind its own copy
                        if st != 4:
                            nc.tensor.matmul(
                                chans[idx], ones_t[:], bias_t[0:1, n0:n1],
                                start=False, stop=True,
                            )
                        if n0 == 0:
                            nc.vector.tensor_copy(osb[:, n0:n1], chans[idx])
                        else:
                            nc.scalar.copy(osb[:, n0:n1], chans[idx])
                        if st in (4, 7) or n0 != 0:
                            # st4/st7 flush per-chunk (earliest start /
                            # smallest finish); st5/st6 flush once complete
                            w0 = 0 if (st in (5, 6)) else n0
                            nc.sync.dma_start(
                                out[T0 + st * 128 : T0 + (st + 1) * 128,
                                    w0:n1],
                                osb[:, w0:n1],
                            )

            def drain(gen):
                for g in gen:
                    g()

            # head: frame 0's full qkv + rope (PE-heavy, ACT idle)
            make_frame_tiles(0)
            drain(qkv_groups(0, "head"))
            nc.scalar.dma_start(cos_t[:], cosP[:])
            nc.scalar.dma_start(sin_t[:], sinP[:])
            nc.scalar.dma_start(wo6_t[:], wo6d[:])
            nc.scalar.dma_start(bias_t[:], bias_rep[:])
            drain(qkv_groups(0, (0, 1, 2, 3, 4, 5)))

            # prefetch frame 1's first x chunk before attention(0) starts
            # pulling filler, so the first frame-1 v matmul never waits on
            # its DMA; scalar queue = after the weight stream, clear of the
            # startup HWDGE contention
            make_frame_tiles(1)
            emit_x(1, 0, eng=nc.scalar)

            # attention(0), qc-major, with frame 1's qkv+rope for pairs 0-3
            # as filler; pairs 4-5 are deferred into the ACT-bound
            # attention(1) qc=0 span to balance PE load across phases
            filler1 = itertools.chain(
                qkv_groups(1, "head", skip_x0=True), qkv_groups(1, (0, 1, 2, 3))
            )
            for qc in range(NQC):
                for p in range(NPAIR):
                    attn_block(0, p, qc, filler1)
            drain(filler1)

            # attention(1) qc=0: frame-1 pairs 4-5 prep + outproj(0)
            filler2a = itertools.chain(
                qkv_groups(1, (4, 5)), outproj_groups(0, sts=range(4))
            )
            for p in range(NPAIR):
                attn_block(1, p, 0, filler2a)
            drain(filler2a)

            # all qkv projections done: swap the weight pool for a deep osb
            # pool so the remaining outproj/tail flushes never stall on
            # buffer rotation
            w6p_pool.release()
            pools["osb"] = tc.alloc_tile_pool(name="osbb", bufs=6)

            # attention(1) qc=1 interleaved with the first half of outproj(1)
            # (its qc=0 ot columns are complete by now) and the tail wave's
            # j0-3 chains
            filler2b = itertools.chain(
                outproj_groups(0, sts=range(4, KT)),
                outproj_groups(1, sts=range(4)),
            )
            for p in range(NPAIR):
                attn_block(1, p, 1, filler2b, last=(p == NPAIR - 1))
            drain(filler2b)

            # tail: wave-ordered outproj(1) st4-7
            tail_wave()
            pools["osb"].release()

    nc.compile()
    return nc


def _prep_inputs(x, w_qkv, w_out, b_out, H, W):
    """Host-side prep: shard + transpose + bf16 cast. Per-core in_maps."""
    S = H * W
    SL = FPC * S
    nframes = x.shape[0] * x.shape[1]
    ncores = nframes // FPC
    xf = np.asarray(x, dtype=np.float32).reshape(nframes, S, D)

    wqkvT = np.ascontiguousarray(np.asarray(w_qkv, np.float32).T)  # [768, 2304]
    # permute each q/k head's 64 output dims into [even|odd] blocks so
    # rotate-half is a partition-block swap (scores are invariant to any
    # per-head dim permutation applied to both q and k)
    wqkvT = wqkvT.copy()
    for part in range(2):  # q, k
        for h in range(NH):
            c0 = part * D + h * HD
            wqkvT[:, c0 : c0 + HD] = wqkvT[:, c0 + PERM64]
    w_outT = np.ascontiguousarray(np.asarray(w_out, np.float32).T)  # [768, 768]
    # pack 128-row blocks per partition: t6[p, j, e] = tT[128j + p, e]
    w6 = wqkvT.reshape(6, 128, 3 * D).transpose(1, 0, 2).reshape(128, -1).astype(BF16)
    wo6 = w_outT.reshape(6, 128, D).transpose(1, 0, 2).reshape(128, -1).astype(BF16)
    cos, sin = _host_rope(H, W, HD)  # [S, 64]
    cosB = cos.T[PERM64]  # [64, S] permuted rows
    # sign folded into sin: new rows 0-31 (even dims) need -sin
    sgn = np.concatenate([-np.ones(32), np.ones(32)]).astype(np.float32)
    sinB = sgn[:, None] * sin.T[PERM64]
    cosP = np.tile(cosB, (2, 1)).astype(BF16)  # [128, S]
    sinP = np.tile(sinB, (2, 1)).astype(BF16)
    bias_rep = np.tile(np.asarray(b_out, np.float32)[None, :], (128, 1)).astype(BF16)

    in_maps = []
    for c in range(ncores):
        shard = xf[c * FPC : (c + 1) * FPC].reshape(SL, D)
        xT = np.ascontiguousarray(shard.T)  # [768, SL]
        x6 = xT.reshape(6, 128, SL).transpose(1, 0, 2).reshape(128, -1).astype(BF16)
        in_maps.append(
            dict(
                x6=x6,
                w6=w6,
                wo6=wo6,
                cosP=cosP,
                sinP=sinP,
                bias_rep=bias_rep,
            )
        )
    return in_maps


def run(x, w_qkv, w_out, b_out, trace=False):
    from concourse import bass_utils

    Hd, Wd = x.shape[2], x.shape[3]
    key = (Hd, Wd)
    if key not in _built:
        _built[key] = build_nc(Hd, Wd)
    nc = _built[key]
    in_maps = _prep_inputs(x, w_qkv, w_out, b_out, Hd, Wd)
    res = bass_utils.run_bass_kernel_spmd(
        nc, in_maps, core_ids=list(range(len(in_maps))), trace=trace
    )
    outs = [r["out"] for r in res.results]
    full = np.concatenate(outs, axis=0).reshape(B, T, Hd, Wd, D).astype(np.float32)
    return full, res


def kernel(x, w_qkv, w_out, b_out):
    full, _ = run(x, w_qkv, w_out, b_out, trace=False)
    return full


# revision 88
# speedup vs baseline: 1.0003x; 1.0003x over previous
"""Trainium2 Bass kernel for nn_Attention_91225105367483.

Spatial attention: x[B=2,T=8,H=32,W=32,D=768] -> 16 frames x 1024 tokens.
Data-parallel over frames: 8 cores x 2 frames each. No collectives.

v5 design (v3 + PE offloads + pipeline/tail restructuring; 344us -> 306us
in timeline-sim):
  - all matmuls bf16 (fp8 measured too coarse for the 2% gate).
  - scores transposed [keys, queries], 2 heads packed into the PE array via
    tile_position quadrants (64-row contraction each).
  - exp on ACT with scale=1/8, bias=-2 (softmax-invariant shift).
  - softmax denominators ride row 64 of the AV matmul (ones column in v);
    normalize: den row -> [1,QCS] copy -> reciprocal -> GPSIMD
    partition_broadcast -> two aligned per-head muls. No PE involvement.
    The last block's evac copies go to ACT (idle then) off the DVE chain.
  - RoPE: q/k head dims are host-permuted into [even(32)|odd(32)] blocks
    per head (score contraction is permutation-invariant), so rotate-half
    becomes four partition-block swap DMAs per tile; the sign lives in the
    sin table. The cos-mul runs on idle GPSIMD, sin-mul + add on DVE.
  - emission interleaves frame f+1's QKV/RoPE into frame f's attention
    blocks so the in-order PE queue always has ready work while ACT runs
    exp; AV lags its exp by 5 key tiles so a not-yet-ready AV never
    head-of-line-blocks the PE queue.
  - cold start: x/w DMAs are chunked and ordered against the serial
    HWDGE/DMA line (x j0,j1 small first; per-j wv/wq; wk 2x3j; cos/sin/
    wo6/bias deferred past the head phase); the chunk-0 v projection is
    emitted j-major across 8 borrowed PSUM regions so each (x_j, wv_j)
    arrival feeds 3072 output cols; separate v/q/k weight tiles keep
    reads from over-waiting on unrelated weight writes; frame-0 qk evacs
    run on ACT (DVE would backlog and hold PSUM banks).
  - tail: outproj(1) st4-7 runs as 8 parallel chains over all PSUM banks
    with the j=5 (last pair) matmul deferred to the end of each chain, the
    output bias folded in as a K=1 ones-row matmul, evac copies split
    DVE/ACT, and one merged flush DMA per s-tile; the weight pool is
    released mid-kernel (LIFO heap) to fund a 6-deep osb pool so flushes
    never stall on buffer rotation.
"""
import sys

sys.path.insert(0, "/opt/trn_rl_repo")

import numpy as np
import ml_dtypes

BF16 = ml_dtypes.bfloat16

B, T, D = 2, 8, 768
NH, HD = 12, 64
NCORES = 8
FPC = 2  # frames per core
NPAIR = NH // 2

GP_ROPE = True  # offload rope cos-mul to GPSIMD

_built = {}


def _host_rope(H, W, head_dim):
    """Replicates reference._rope_cos_sin in numpy fp32."""
    half = head_dim // 4
    inv_freq = (1.0 / (10000.0 ** (np.arange(half, dtype=np.float32) / half))).astype(
        np.float32
    )
    th_h = np.arange(H, dtype=np.float32)[:, None] * inv_freq  # [H, half]
    th_w = np.arange(W, dtype=np.float32)[:, None] * inv_freq  # [W, half]
    cos = np.concatenate(
        [
            np.broadcast_to(np.cos(th_h)[:, None, :], (H, W, half)),
            np.broadcast_to(np.cos(th_w)[None, :, :], (H, W, half)),
        ],
        axis=-1,
    )
    sin = np.concatenate(
        [
            np.broadcast_to(np.sin(th_h)[:, None, :], (H, W, half)),
            np.broadcast_to(np.sin(th_w)[None, :, :], (H, W, half)),
        ],
        axis=-1,
    )
    cos = np.repeat(cos, 2, axis=-1).reshape(H * W, head_dim).astype(np.float32)
    sin = np.repeat(sin, 2, axis=-1).reshape(H * W, head_dim).astype(np.float32)
    return cos, sin


# new row i<32 holds old (even) dim 2i; row 32+i holds old (odd) dim 2i+1
PERM64 = np.concatenate([np.arange(0, 64, 2), np.arange(1, 64, 2)])


def build_nc(H, W):
    """Builds the per-core Bass program. S = H*W tokens per frame."""
    import concourse.bass as bass
    import concourse.tile as tile
    from concourse import bacc, mybir

    dt = mybir.dt
    ActF = mybir.ActivationFunctionType
    S = H * W  # 1024 tokens per frame
    SL = FPC * S  # 2048 tokens per core
    QCS = 512  # query chunk
    NQC = S // QCS  # 2
    KT = S // 128  # 8 key tiles per frame
    NDC = D // 128  # 6 contraction chunks

    nc = bacc.Bacc("TRN2", target_bir_lowering=False, debug=False)

    x6d = nc.dram_tensor("x6", [128, NDC * SL], dt.bfloat16, kind="ExternalInput")
    w6d = nc.dram_tensor("w6", [128, NDC * 3 * D], dt.bfloat16, kind="ExternalInput")
    wo6d = nc.dram_tensor("wo6", [128, NDC * D], dt.bfloat16, kind="ExternalInput")
    cosP = nc.dram_tensor("cosP", [128, S], dt.bfloat16, kind="ExternalInput")
    sinP = nc.dram_tensor("sinP", [128, S], dt.bfloat16, kind="ExternalInput")
    bias_rep = nc.dram_tensor("bias_rep", [128, D], dt.bfloat16, kind="ExternalInput")
    out = nc.dram_tensor("out", [SL, D], dt.float32, kind="ExternalOutput")

    with tile.TileContext(nc) as tc:
        import contextlib
        import itertools

        ctx = contextlib.ExitStack()
        with ctx:
            const = ctx.enter_context(tc.tile_pool(name="const", bufs=1))
            xt_pool = ctx.enter_context(tc.tile_pool(name="xt", bufs=3))
            qk_pool = ctx.enter_context(tc.tile_pool(name="qk", bufs=1))
            swp_pool = ctx.enter_context(tc.tile_pool(name="swp", bufs=1))
            v_pool = ctx.enter_context(tc.tile_pool(name="v", bufs=1))
            ot_pool = ctx.enter_context(tc.tile_pool(name="ot", bufs=1))
            et_pool = ctx.enter_context(tc.tile_pool(name="et", bufs=7))
            rtmp_pool = ctx.enter_context(tc.tile_pool(name="rtmp", bufs=4))
            nrm_pool = ctx.enter_context(tc.tile_pool(name="nrm", bufs=2))
            osba_pool = ctx.enter_context(tc.tile_pool(name="osba", bufs=2))
            mm_ps = ctx.enter_context(tc.tile_pool(name="mmps", bufs=2, space="PSUM"))
            sc_ps = ctx.enter_context(tc.tile_pool(name="scps", bufs=2, space="PSUM"))
            av_ps = ctx.enter_context(tc.tile_pool(name="avps", bufs=2, space="PSUM"))
            # qkv weights live in a manually-released pool: dead after the
            # last frame-1 projection, their 27.6KB then funds a deep osb
            # pool for the outproj/tail flushes (pool heap is LIFO)
            w6p_pool = tc.alloc_tile_pool(name="w6p", bufs=1)
            pools = {"osb": osba_pool}

            frames = [{} for _ in range(FPC)]
            x6v = x6d[:].rearrange("p (j t) -> p j t", j=NDC)

            # ---- constants ----
            # DMA order matters: HWDGE serializes all DMAs (~625ns apiece)
            # and DMA_ENGINES serializes transfers, so arrivals must track PE
            # consumption: (x_j, wv_j) pairs first, then per-j wq/wk, then
            # cos/sin, wo6/bias last.
            xt00 = xt_pool.tile([128, NDC * QCS], dt.bfloat16, tag="xt",
                                name="xt_0_0")
            frames[0]["xt0"] = xt00
            # separate v/q/k weight tiles: a read then only depends on its
            # own tile's DMAs (a single shared tile made every qk chain wait
            # for the LAST of the 14 weight writes)
            w6v_t = w6p_pool.tile([128, NDC * D], dt.bfloat16, tag="w6v")
            w6q_t = w6p_pool.tile([128, NDC * D], dt.bfloat16, tag="w6q")
            w6k_t = w6p_pool.tile([128, NDC * D], dt.bfloat16, tag="w6k")
            w63s = w6d[:].rearrange("p (j e) -> p j e", j=NDC)
            w6vv = w6v_t[:].rearrange("p (j e) -> p j e", j=NDC)
            w6qv = w6q_t[:].rearrange("p (j e) -> p j e", j=NDC)
            w6kv = w6k_t[:].rearrange("p (j e) -> p j e", j=NDC)
            # x chunk 0 as j0, j1, then j2-5 merged: the first matmul waits
            # only on a small j0 transfer, later j's amortize HWDGE slots
            xt00v = xt00[:].rearrange("p (j t) -> p j t", j=NDC)
            for h0, hn in ((0, 1), (1, 1), (2, 4)):
                nc.sync.dma_start(
                    xt00v[:, h0 : h0 + hn, :], x6v[:, h0 : h0 + hn, 0:QCS]
                )
            for j in range(NDC):
                nc.scalar.dma_start(
                    w6vv[:, j, :], w63s[:, j, 2 * D : 3 * D]
                )
            for h0 in (0, 2, 4):  # q columns in 2j chunks (consumed later)
                nc.scalar.dma_start(
                    w6qv[:, h0 : h0 + 2, :], w63s[:, h0 : h0 + 2, 0:D]
                )
            for h0 in (0, 3):  # k columns in two 3-j chunks (consumed later)
                nc.scalar.dma_start(
                    w6kv[:, h0 : h0 + 3, :],
                    w63s[:, h0 : h0 + 3, D : 2 * D],
                )
            # cos/sin/wo6/bias DMAs are deferred until after the head phase
            # so their transfers don't delay wq/wk in the serial DMA line
            cos_t = const.tile([128, S], dt.bfloat16, tag="cos")
            sin_t = const.tile([128, S], dt.bfloat16, tag="sin")
            wo6_t = const.tile([128, NDC * D], dt.bfloat16, tag="wo6")
            bias_t = const.tile([128, D], dt.bfloat16, tag="bias")
            # K=1 ones row: folds the output bias into each tail outproj
            # chain as one extra matmul (PE idles at the tail; DVE doesn't)
            ones_t = const.tile([1, 128], dt.bfloat16, tag="onesr")
            nc.vector.memset(ones_t[:], 1.0)

            # per-partition bias operand for exp(s*scale - 2): keeps et well
            # inside bf16 range; softmax ratio is invariant to the shift
            expb_t = const.tile([128, 1], dt.float32, tag="expb")
            nc.vector.memset(expb_t[:], -2.0)

            wo6 = wo6_t[:].rearrange("p (j e) -> p j e", j=NDC)

            scale = 1.0 / np.sqrt(HD)

            def make_frame_tiles(f):
                fr = frames[f]
                fr["qk_q"] = [
                    qk_pool.tile([128, S], dt.bfloat16, tag=f"q{p}", bufs=2,
                                 name=f"qkq{f}_{p}")
                    for p in range(NPAIR)
                ]
                fr["qk_k"] = [
                    qk_pool.tile([128, S], dt.bfloat16, tag=f"k{p}", bufs=2,
                                 name=f"qkk{f}_{p}")
                    for p in range(NPAIR)
                ]
                fr["v_sb"] = [
                    v_pool.tile([128, NH * 65], dt.bfloat16, tag=f"v{i}", bufs=2,
                                name=f"vsb{f}_{i}")
                    for i in range(KT)
                ]
                fr["ot"] = [
                    ot_pool.tile([128, S], dt.bfloat16, tag=f"ot{d}", bufs=2,
                                 name=f"ot{f}_{d}")
                    for d in range(NDC)
                ]
                for i in range(KT):
                    vv = fr["v_sb"][i][:].rearrange("p (h c) -> p h c", h=NH)
                    nc.vector.memset(vv[:, :, HD : HD + 1], 1.0)

            # ---- emission-level software pipeline ----

            def emit_x(f, c, eng=None):
                """One x-chunk DMA on a queue that won't contend with the
                startup weight stream (HWDGE serializes all DMAs)."""
                fr = frames[f]
                xt = xt_pool.tile([128, NDC * QCS], dt.bfloat16, tag="xt",
                                  name=f"xt_{f}_{c}")
                t0 = f * S + c * QCS
                (eng or nc.sync).dma_start(
                    xt[:].rearrange("p (j t) -> p j t", j=NDC),
                    x6v[:, :, t0 : t0 + QCS],
                )
                fr[f"xt{c}"] = xt

            def qkv_groups(f, part, skip_x0=False):
                """Yields closures, one matmul-group each, for frame f's qkv
                projection (v in natural layout, q/k transposed). part="head"
                yields x DMAs + v + pair 0's q/k/rope (everything the first
                attention block of the frame needs); part="rest" yields pairs
                1..5, each pair's q/k followed by its swap DMAs + rope."""

                def mk_v(c, st):
                    def emit_v(c=c, st=st, f=f):
                        fr = frames[f]
                        kt = c * 4 + st
                        x3 = fr[f"xt{c}"][:].rearrange("p (j t) -> p j t", j=NDC)
                        for nch in range(2):
                            n0 = nch * 512
                            nw = 512 - nch * 256  # v cols: 512 + 256
                            ps = mm_ps.tile([128, 512], dt.float32, tag="mm",
                                            name=f"vps_{f}_{kt}_{nch}")
                            for j in range(NDC):
                                nc.tensor.matmul(
                                    ps[:, 0:nw],
                                    x3[:, j, st * 128 : (st + 1) * 128],
                                    w6vv[:, j, n0 : n0 + nw],
                                    start=(j == 0),
                                    stop=(j == NDC - 1),
                                )
                            vh = fr["v_sb"][kt][:].rearrange(
                                "p (h c) -> p h c", h=NH
                            )
                            pv = ps[:, 0:nw].rearrange("p (h c) -> p h c", c=HD)
                            h0 = nch * 8
                            nh = 8 - nch * 4
                            if f == 0:
                                nc.scalar.copy(vh[:, h0 : h0 + nh, 0:HD], pv[:])
                            else:
                                nc.vector.tensor_copy(
                                    vh[:, h0 : h0 + nh, 0:HD], pv[:])

                    return emit_v

                def mk_qk(c, et):
                    def emit_qk(c=c, et=et, f=f):
                        fr = frames[f]
                        x3 = fr[f"xt{c}"][:].rearrange("p (j t) -> p j t", j=NDC)
                        ps = mm_ps.tile([128, 512], dt.float32, tag="mm",
                                        name=f"qkps_{f}_{c}_{et}")
                        wsl = w6qv if et < 6 else w6kv
                        e0 = (et % 6) * 128
                        for j in range(NDC):
                            nc.tensor.matmul(
                                ps[:],
                                wsl[:, j, e0 : e0 + 128],
                                x3[:, j, :],
                                start=(j == 0),
                                stop=(j == NDC - 1),
                            )
                        dst = fr["qk_q"][et] if et < 6 else fr["qk_k"][et - 6]
                        if f == 0:
                            nc.scalar.copy(dst[:, c * QCS : (c + 1) * QCS], ps[:])
                        else:
                            nc.vector.tensor_copy(
                                dst[:, c * QCS : (c + 1) * QCS], ps[:])

                    return emit_qk

                def emit_v_cold(f=f):
                    """Cold-start v projection for chunk 0, j-major across
                    eight PSUM regions (borrowing the idle av and sc banks):
                    each (x_j, wv_j) DMA arrival feeds all 8 chain steps
                    (3072 output cols ~= 1.28us), matching PE consumption to
                    the ~1.25us/pair serial HWDGE cadence."""
                    fr = frames[f]
                    x3 = fr["xt0"][:].rearrange("p (j t) -> p j t", j=NDC)
                    ps_n0 = [
                        mm_ps.tile([128, 512], dt.float32, tag="mm",
                                   name=f"vcold_m{st}")
                        for st in range(2)
                    ] + [
                        av_ps.tile([128, 512], dt.float32, tag="av",
                                   name=f"vcold_a{st}")
                        for st in range(2)
                    ]
                    sc_a = sc_ps.tile([128, 2 * QCS], dt.float32, tag="sc",
                                      name="vcold_sc0")
                    sc_b = sc_ps.tile([128, 2 * QCS], dt.float32, tag="sc",
                                      name="vcold_sc1")
                    ps_n1 = [sc_a[:, 0:256], sc_a[:, 512:768],
                             sc_b[:, 0:256], sc_b[:, 512:768]]
                    for j in range(NDC):
                        for st in range(4):
                            nc.tensor.matmul(
                                ps_n0[st][:],
                                x3[:, j, st * 128 : (st + 1) * 128],
                                w6vv[:, j, 0:512],
                                start=(j == 0),
                                stop=(j == NDC - 1),
                            )
                        for st in range(4):
                            nc.tensor.matmul(
                                ps_n1[st],
                                x3[:, j, st * 128 : (st + 1) * 128],
                                w6vv[:, j, 512:D],
                                start=(j == 0),
                                stop=(j == NDC - 1),
                            )
                    for st in range(4):
                        vh = fr["v_sb"][st][:].rearrange("p (h c) -> p h c", h=NH)
                        pv = ps_n0[st][:].rearrange("p (h c) -> p h c", c=HD)
                        nc.vector.tensor_copy(vh[:, 0:8, 0:HD], pv[:])
                        pv1 = ps_n1[st].rearrange("p (h c) -> p h c", c=HD)
                        nc.vector.tensor_copy(vh[:, 8:12, 0:HD], pv1[:])

                if part == "head":
                    # chunk 0: x, v, all 12 q/k tiles; chunk 1: x, v
                    if f != 0 and not skip_x0:
                        yield lambda: emit_x(f, 0)
                    if f == 0:
                        yield emit_v_cold
                    else:
                        for st in range(4):
                            yield mk_v(0, st)
                    for et in range(12):
                        yield mk_qk(0, et)
                    yield lambda: emit_x(f, 1, eng=nc.scalar)
                    for st in range(4):
                        yield mk_v(1, st)
                else:
                    # chunk 1 q/k with each pair's swap + rope woven in after
                    for p in part:
                        yield mk_qk(1, p)
                        yield mk_qk(1, 6 + p)
                        yield from rope_groups(f, pairs=[p])

            def rope_groups(f, pairs=None):
                """Yields closures: first the pair's rotate-half swap DMAs,
                then one rope chunk (pair, q-or-k, half-frame) each."""
                for p in pairs if pairs is not None else range(NPAIR):
                    def emit_swap(p=p, f=f):
                        fr = frames[f]
                        for which in range(2):
                            tens = (fr["qk_q"] if which == 0 else fr["qk_k"])[p]
                            swp = swp_pool.tile([128, S], dt.bfloat16,
                                                tag="sw", bufs=2,
                                                name=f"swp_{f}_{p}_{which}")
                            fr[f"swp{which}_{p}"] = swp
                            for b0, b1 in ((0, 32), (64, 96)):
                                nc.sync.dma_start(
                                    swp[b0 : b0 + 32, :], tens[b1 : b1 + 32, :]
                                )
                                nc.sync.dma_start(
                                    swp[b1 : b1 + 32, :], tens[b0 : b0 + 32, :]
                                )

                    yield emit_swap
                    for which in range(2):
                        for c in range(2):
                            def emit_rope(p=p, which=which, c=c, f=f):
                                fr = frames[f]
                                tens = (fr["qk_q"] if which == 0 else fr["qk_k"])[p]
                                swp = fr[f"swp{which}_{p}"]
                                sl_ = slice(c * 512, (c + 1) * 512)
                                t1 = rtmp_pool.tile([128, 512], dt.bfloat16, tag="rt1",
                                                    name=f"rt1_{f}_{p}_{which}_{c}")
                                nc.vector.tensor_mul(t1[:], swp[:, sl_], sin_t[:, sl_])
                                t2 = rtmp_pool.tile([128, 512], dt.bfloat16, tag="rt2",
                                                    name=f"rt2_{f}_{p}_{which}_{c}")
                                # t2 is off the critical path -> idle GPSIMD;
                                # the add gates scores -> keep it on DVE
                                eng = nc.gpsimd if GP_ROPE else nc.vector
                                eng.tensor_mul(t2[:], tens[:, sl_], cos_t[:, sl_])
                                nc.vector.tensor_add(tens[:, sl_], t1[:], t2[:])

                            yield emit_rope

            def attn_block(f, p, qc, filler, last=False):
                """Emits attention for (frame, pair, query-chunk), pulling
                groups from `filler` so the PE queue has independent work.
                For the final block the PSUM-evac copies go to the idle ACT
                engine so the tail-gating normalize isn't DVE-serial."""
                fr = frames[f]
                qsl = slice(qc * QCS, (qc + 1) * QCS)
                avp = [
                    av_ps.tile([128, QCS], dt.float32, tag="av",
                               name=f"avp_{f}_{p}_{qc}_{hh}")
                    for hh in range(2)
                ]

                def emit_av(kt, et_t):
                    for hh in range(2):
                        h = 2 * p + hh
                        nc.tensor.matmul(
                            avp[hh][0:65, :],
                            fr["v_sb"][kt][:, h * 65 : h * 65 + 65],
                            et_t[:, hh * QCS : (hh + 1) * QCS],
                            start=(kt == 0),
                            stop=(kt == KT - 1),
                        )

                pend = []  # AV lags five key tiles behind exp (queue-head safety)
                for kt in range(KT):
                    et_t = et_pool.tile([128, 2 * QCS], dt.bfloat16, tag="et",
                                        name=f"et_{f}_{p}_{qc}_{kt}")
                    ksl = slice(kt * 128, (kt + 1) * 128)
                    sp = sc_ps.tile([128, 2 * QCS], dt.float32, tag="sc",
                                    name=f"sp_{f}_{p}_{qc}_{kt}")
                    for hh in range(2):
                        rb = 64 * hh
                        nc.tensor.matmul(
                            sp[:, hh * QCS : (hh + 1) * QCS],
                            fr["qk_k"][p][rb : rb + 64, ksl],
                            fr["qk_q"][p][rb : rb + 64, qsl],
                            start=True,
                            stop=True,
                            tile_position=(rb, 0),
                        )
                    nc.scalar.activation(
                        et_t[:], sp[:], ActF.Exp, bias=expb_t[:], scale=float(scale)
                    )
                    pend.append((kt, et_t))
                    if len(pend) > 5:
                        emit_av(*pend.pop(0))
                        g = next(filler, None)
                        if g:
                            g()
                for item in pend:
                    emit_av(*item)

                # ---- normalize: o[0:64] / o[64] for both heads ----
                # evacuate the av PSUM banks immediately (den rows + o rows to
                # SBUF); reciprocal on the [1,QCS] row, GPSIMD broadcasts it
                # across partitions, then one aligned mul per head.
                ou = nrm_pool.tile([128, QCS], dt.bfloat16, tag="ou",
                                   name=f"ou_{f}_{p}_{qc}")
                rbcs = []
                for hh in range(2):
                    d_t = nrm_pool.tile([1, QCS], dt.float32, tag=f"den{hh}",
                                        name=f"den_{f}_{p}_{qc}_{hh}")
                    if last:
                        nc.scalar.copy(d_t[:], avp[hh][64:65, :])
                    else:
                        nc.vector.tensor_copy(d_t[:], avp[hh][64:65, :])
                    nc.vector.reciprocal_approx_fast(d_t[:], d_t[:])
                    rbc = nrm_pool.tile([128, QCS], dt.float32, tag=f"rbc{hh}",
                                        name=f"rbc_{f}_{p}_{qc}_{hh}")
                    nc.gpsimd.partition_broadcast(rbc[:], d_t[:], channels=128)
                    rbcs.append(rbc)
                    if last:
                        nc.scalar.copy(
                            ou[64 * hh : 64 * hh + 64, :], avp[hh][0:64, :]
                        )
                    else:
                        nc.vector.tensor_copy(
                            ou[64 * hh : 64 * hh + 64, :], avp[hh][0:64, :]
                        )
                otd = fr["ot"][p]
                nc.vector.tensor_mul(
                    otd[0:64, qsl], ou[0:64, :], rbcs[0][0:64, :]
                )
                nc.vector.tensor_mul(
                    otd[64:128, qsl], ou[64:128, :], rbcs[1][64:128, :]
                )

            def op_flush(f, st, osb):
                """One merged out-DMA per s-tile (HWDGE serializes DMAs, so
                fewer/bigger beats per-chunk at the tail)."""
                T0 = f * S
                nc.sync.dma_start(
                    out[T0 + st * 128 : T0 + (st + 1) * 128, :], osb[:]
                )

            def op_osb(f, st):
                return pools["osb"].tile([128, D], dt.float32, tag="osb",
                                         name=f"osb_{f}_{st}")

            def outproj_groups(f, sts=None):
                """Yields closures: one out-projection column chunk per group
                (finer granularity spreads filler over more pull slots)."""
                sts = list(sts if sts is not None else range(KT))
                for st in sts:
                    box = {}

                    def mk_chunk(st, n0, n1, box, last):
                        def emit_chunk(st=st, n0=n0, n1=n1, f=f):
                            fr = frames[f]
                            if "osb" not in box:
                                box["osb"] = op_osb(f, st)
                            ps = mm_ps.tile([128, 512], dt.float32, tag="mm",
                                            name=f"ops_{f}_{st}_{n0}")
                            for j in range(NDC):
                                nc.tensor.matmul(
                                    ps[:, : n1 - n0],
                                    fr["ot"][j][:, st * 128 : (st + 1) * 128],
                                    wo6[:, j, n0:n1],
                                    start=(j == 0),
                                    stop=(j == NDC - 1),
                                )
                            nc.vector.tensor_add(
                                box["osb"][:, n0:n1], ps[:, : n1 - n0],
                                bias_t[:, n0:n1],
                            )
                            if last:
                                op_flush(f, st, box["osb"])

                        return emit_chunk

                    yield mk_chunk(st, 0, 512, box, False)
                    yield mk_chunk(st, 512, D, box, True)

            def tail_wave():
                """outproj(1) st 4-7 at drain time. The j contraction runs
                over pair index, so j0-4 of all eight chains (2 mm + 2x2 sc
                + 2 av PSUM regions) execute while the last pair's normalize
                is still in flight; each chain's j=5 then completes with its
                evac immediately behind it so the out-DMAs pipeline under
                the remaining PE work."""
                fr = frames[1]
                specs = [(4, 0, 512), (4, 512, D), (5, 0, 512), (5, 512, D),
                         (6, 0, 512), (6, 512, D), (7, 0, 512), (7, 512, D)]
                chans = []
                sc_tiles = {}
                for st, n0, n1 in specs:
                    if st == 4:
                        ps = mm_ps.tile([128, 512], dt.float32, tag="mm",
                                        name=f"tw_{st}_{n0}")[:, : n1 - n0]
                    elif st == 7:
                        # av banks: their WAR (last block's ou copies) clears
                        # early in the wave, and st7's j5 gates on the same
                        # normalize anyway
                        ps = av_ps.tile([128, QCS], dt.float32, tag="av",
                                        name=f"tw_{st}_{n0}")[:, : n1 - n0]
                    else:
                        if st not in sc_tiles:
                            sc_tiles[st] = sc_ps.tile(
                                [128, 2 * QCS], dt.float32, tag="sc",
                                name=f"twsc_{st}")
                        ps = sc_tiles[st][:, n0 : n0 + (n1 - n0)]
                    chans.append(ps)
                # st7's av banks have a WAR on the last block's norm copies:
                # emit those chains after the other 30 matmuls so the park
                # never reaches the queue head while the WAR is live
                for j in range(5):
                    for idx, (st, n0, n1) in enumerate(specs):
                        if st == 7:
                            continue
                        nc.tensor.matmul(
                            chans[idx],
                            fr["ot"][j][:, st * 128 : (st + 1) * 128],
                            wo6[:, j, n0:n1],
                            start=(j == 0),
                            stop=False,
                        )
                for j in range(5):
                    for idx, (st, n0, n1) in enumerate(specs):
                        if st != 7:
                            continue
                        nc.tensor.matmul(
                            chans[idx],
                            fr["ot"][j][:, st * 128 : (st + 1) * 128],
                            wo6[:, j, n0:n1],
                            start=(j == 0),
                            stop=False,
                        )
                T0 = S
                for st in (4, 5, 6, 7):
                    osb = op_osb(1, st)
                    for idx, (st_, n0, n1) in enumerate(specs):
                        if st_ != st:
                            continue
                        nc.tensor.matmul(
                            chans[idx],
                            fr["ot"][5][:, st * 128 : (st + 1) * 128],
                            wo6[:, 5, n0:n1],
                            start=False,
                            stop=False,
                        )
                        # bias via a K=1 ones-row matmul, then evacuation
                        # copies alternating DVE / idle ACT; each chunk's
                        # flush DMA issues right behind its own copy so the
                        # serial transfer line starts as early as possible
                        nc.tensor.matmul(
                            chans[idx], ones_t[:], bias_t[0:1, n0:n1],
                            start=False, stop=True,
                        )
                        if n0 == 0:
                            nc.vector.tensor_copy(osb[:, n0:n1], chans[idx])
                        else:
                            nc.scalar.copy(osb[:, n0:n1], chans[idx])
                        if st in (4, 7) or n0 != 0:
                            # st4/st7 flush per-chunk (earliest start /
                            # smallest finish); st5/st6 flush once complete
                            w0 = 0 if (st in (5, 6)) else n0
                            nc.sync.dma_start(
                                out[T0 + st * 128 : T0 + (st + 1) * 128,
                                    w0:n1],
                                osb[:, w0:n1],
                            )

            def drain(gen):
                for g in gen:
                    g()

            # head: frame 0's full qkv + rope (PE-heavy, ACT idle)
            make_frame_tiles(0)
            drain(qkv_groups(0, "head"))
            nc.scalar.dma_start(cos_t[:], cosP[:])
            nc.scalar.dma_start(sin_t[:], sinP[:])
            nc.scalar.dma_start(wo6_t[:], wo6d[:])
            nc.scalar.dma_start(bias_t[:], bias_rep[:])
            drain(qkv_groups(0, (0, 1, 2, 3, 4, 5)))

            # prefetch frame 1's first x chunk before attention(0) starts
            # pulling filler, so the first frame-1 v matmul never waits on
            # its DMA; scalar queue = after the weight stream, clear of the
            # startup HWDGE contention
            make_frame_tiles(1)
            emit_x(1, 0, eng=nc.scalar)

            # attention(0), qc-major, with frame 1's qkv+rope for pairs 0-3
            # as filler; pairs 4-5 are deferred into the ACT-bound
            # attention(1) qc=0 span to balance PE load across phases
            filler1 = itertools.chain(
                qkv_groups(1, "head", skip_x0=True), qkv_groups(1, (0, 1, 2, 3))
            )
            for qc in range(NQC):
                for p in range(NPAIR):
                    attn_block(0, p, qc, filler1)
            drain(filler1)

            # attention(1) qc=0: frame-1 pairs 4-5 prep + outproj(0)
            filler2a = itertools.chain(
                qkv_groups(1, (4, 5)), outproj_groups(0, sts=range(4))
            )
            for p in range(NPAIR):
                attn_block(1, p, 0, filler2a)
            drain(filler2a)

            # all qkv projections done: swap the weight pool for a deep osb
            # pool so the remaining outproj/tail flushes never stall on
            # buffer rotation
            w6p_pool.release()
            pools["osb"] = tc.alloc_tile_pool(name="osbb", bufs=6)

            # attention(1) qc=1 interleaved with the first half of outproj(1)
            # (its qc=0 ot columns are complete by now) and the tail wave's
            # j0-3 chains
            filler2b = itertools.chain(
                outproj_groups(0, sts=range(4, KT)),
                outproj_groups(1, sts=range(4)),
            )
            for p in range(NPAIR):
                attn_block(1, p, 1, filler2b, last=(p == NPAIR - 1))
            drain(filler2b)

            # tail: wave-ordered outproj(1) st4-7
            tail_wave()
            pools["osb"].release()

    nc.compile()
    return nc


def _prep_inputs(x, w_qkv, w_out, b_out, H, W):
    """Host-side prep: shard + transpose + bf16 cast. Per-core in_maps."""
    S = H * W
    SL = FPC * S
    nframes = x.shape[0] * x.shape[1]
    ncores = nframes // FPC
    xf = np.asarray(x, dtype=np.float32).reshape(nframes, S, D)

    wqkvT = np.ascontiguousarray(np.asarray(w_qkv, np.float32).T)  # [768, 2304]
    # permute each q/k head's 64 output dims into [even|odd] blocks so
    # rotate-half is a partition-block swap (scores are invariant to any
    # per-head dim permutation applied to both q and k)
    wqkvT = wqkvT.copy()
    for part in range(2):  # q, k
        for h in range(NH):
            c0 = part * D + h * HD
            wqkvT[:, c0 : c0 + HD] = wqkvT[:, c0 + PERM64]
    w_outT = np.ascontiguousarray(np.asarray(w_out, np.float32).T)  # [768, 768]
    # pack 128-row blocks per partition: t6[p, j, e] = tT[128j + p, e]
    w6 = wqkvT.reshape(6, 128, 3 * D).transpose(1, 0, 2).reshape(128, -1).astype(BF16)
    wo6 = w_outT.reshape(6, 128, D).transpose(1, 0, 2).reshape(128, -1).astype(BF16)
    cos, sin = _host_rope(H, W, HD)  # [S, 64]
    cosB = cos.T[PERM64]  # [64, S] permuted rows
    # sign folded into sin: new rows 0-31 (even dims) need -sin
    sgn = np.concatenate([-np.ones(32), np.ones(32)]).astype(np.float32)
    sinB = sgn[:, None] * sin.T[PERM64]
    cosP = np.tile(cosB, (2, 1)).astype(BF16)  # [128, S]
    sinP = np.tile(sinB, (2, 1)).astype(BF16)
    bias_rep = np.tile(np.asarray(b_out, np.float32)[None, :], (128, 1)).astype(BF16)

    in_maps = []
    for c in range(ncores):
        shard = xf[c * FPC : (c + 1) * FPC].reshape(SL, D)
        xT = np.ascontiguousarray(shard.T)  # [768, SL]
        x6 = xT.reshape(6, 128, SL).transpose(1, 0, 2).reshape(128, -1).astype(BF16)
        in_maps.append(
            dict(
                x6=x6,
                w6=w6,
                wo6=wo6,
                cosP=cosP,
                sinP=sinP,
                bias_rep=bias_rep,
            )
        )
    return in_maps


def run(x, w_qkv, w_out, b_out, trace=False):
    from concourse import bass_utils

    Hd, Wd = x.shape[2], x.shape[3]
    key = (Hd, Wd)
    if key not in _built:
        _built[key] = build_nc(Hd, Wd)
    nc = _built[key]
    in_maps = _prep_inputs(x, w_qkv, w_out, b_out, Hd, Wd)
    res = bass_utils.run_bass_kernel_spmd(
        nc, in_maps, core_ids=list(range(len(in_maps))), trace=trace
    )
    outs = [r["out"] for r in res.results]
    full = np.concatenate(outs, axis=0).reshape(B, T, Hd, Wd, D).astype(np.float32)
    return full, res


def kernel(x, w_qkv, w_out, b_out):
    full, _ = run(x, w_qkv, w_out, b_out, trace=False)
    return full
